# revision 1
# baseline (speedup 1.0000x reference)
"""Trainium2 Bass kernel for BertAlibiUnpadSelfAttention.

Problem shapes (hardcoded): B=2, S=2048, H=12, D=64, DIM=768.
Reference computation:
    qkv = hidden @ Wqkv_w.T + Wqkv_b            # (4096, 2304)
    pad via indices (a permutation -> pure row shuffle)
    q,k,v = split/reshape -> (b, h, s, d)
    scores = q @ k.T / sqrt(64) + bias          # bias dense (2,12,2048,2048)
    attn = softmax(scores) @ v -> (4096, 768), unpad via indices

Sharding: 24 (batch, head) pairs -> 3 per core across 8 cores. Each core
computes its own slice of the QKV projection (disjoint columns/rows -> no
redundant FLOPs) and full attention for its 3 heads.

Device kernel layout choices:
  - qT/kT computed in [d, s] layout directly (lhsT = W slices, rhs = hidden^T),
    which is exactly the layout the scores matmul wants.  1/sqrt(D) folded
    into Wq + bq on the host.
  - scores are computed TRANSPOSED: scoresT[sk, sq] tiles, so the softmax
    reduction (over sk) can be done by the PV matmul itself: V gets an
    appended ones-column, so PV produces [attnT ; sums] in one accumulation.
  - bias is pre-transposed per head on the host; VectorE adds it to the
    score PSUM; ScalarE applies exp (no max subtraction: logits ~ N(0,1),
    fp32 exp is exact-safe here).
  - Final normalize (divide by sums) + transpose back to [s, d] + V-bias add
    happen on the host (tiny: 3x65x2048 per core).
"""

import math
import numpy as np

B, S, H, D = 2, 2048, 12, 64
DIM = H * D            # 768
TOTAL = B * S          # 4096
HPC = 3                # heads per core
N_CORES = 8
KT = DIM // 128        # 6 k-tiles of 128
SQC = S // 512         # 4 free-dim chunks of 512
SKT = S // 128         # 16 sk tiles of 128

_CACHE = {}


def _build_nc(variant="inject"):
    """Build + compile the per-core Bass module.

    All matmuls use tf32 (float32r) operands, fp32 PSUM accumulation.
    The dense additive bias is shipped as fp16 (error ~5e-5, halves DMA).

    variant:
      "inject" - bias tiles are injected into PSUM via an fp16 identity
                 matmul before the QK accumulation; ScalarE exp reads the
                 biased scores straight from PSUM.  Keeps the PE stream
                 dense (HAM-friendly) and VectorE nearly idle.
      "expb"   - host ships exp(bias) instead; scores exp'd from PSUM and
                 multiplied by expb on VectorE.  Fewer PE instructions.
    """
    from concourse import bacc, mybir, tile

    f32 = mybir.dt.float32
    f16 = mybir.dt.float16
    proj_dt = f16
    att_dt = f16

    def mmap(ap):
        return ap

    nc = bacc.Bacc("TRN2", target_bir_lowering=False, debug=False)

    hT = nc.dram_tensor("hT", (DIM, S), proj_dt, kind="ExternalInput")
    wq = nc.dram_tensor("wq", (DIM, HPC * D), proj_dt, kind="ExternalInput")
    wk = nc.dram_tensor("wk", (DIM, HPC * D), proj_dt, kind="ExternalInput")
    wv = nc.dram_tensor("wv", (DIM, HPC * D), proj_dt, kind="ExternalInput")
    bq = nc.dram_tensor("bq", (HPC * D, 1), f32, kind="ExternalInput")
    bk = nc.dram_tensor("bk", (HPC * D, 1), f32, kind="ExternalInput")
    bias_t = nc.dram_tensor("bias_t", (HPC, S, S), f16, kind="ExternalInput")
    ident = nc.dram_tensor("ident", (128, 128), f16, kind="ExternalInput")
    out = nc.dram_tensor("out", (HPC, D + 1, S), f32, kind="ExternalOutput")

    EXP = mybir.ActivationFunctionType.Exp

    with tile.TileContext(nc) as tc:
        with (
            tc.tile_pool(name="const", bufs=1) as constp,
            tc.tile_pool(name="bias", bufs=10) as biasp,
            tc.tile_pool(name="pt", bufs=8) as ptp,
            tc.tile_pool(name="ot", bufs=3) as otp,
        ):
            # ---- load persistent inputs (small tensors first, on the
            # scalar HWDGE queue; hT on the sync queue) ----
            ht = [constp.tile([128, S], proj_dt, tag=f"ht{i}", name=f"ht{i}") for i in range(KT)]
            for i in range(KT):
                nc.sync.dma_start(ht[i][:], hT[i * 128:(i + 1) * 128, :])

            wq_sb = [constp.tile([128, HPC * D], proj_dt, tag=f"wq{i}", name=f"wq{i}") for i in range(KT)]
            wk_sb = [constp.tile([128, HPC * D], proj_dt, tag=f"wk{i}", name=f"wk{i}") for i in range(KT)]
            wv_sb = [constp.tile([128, HPC * D], proj_dt, tag=f"wv{i}", name=f"wv{i}") for i in range(KT)]
            bq_sb = constp.tile([128, 1], f32, tag="bq0")
            bq_sb2 = constp.tile([64, 1], f32, tag="bq1")
            bk_sb = constp.tile([128, 1], f32, tag="bk0")
            bk_sb2 = constp.tile([64, 1], f32, tag="bk1")
            ident_sb = constp.tile([128, 128], f16, tag="ident")
            nc.scalar.dma_start(ident_sb[:], ident[:, :])
            nc.scalar.dma_start(bq_sb[:], bq[0:128, :])
            nc.scalar.dma_start(bq_sb2[:], bq[128:192, :])
            nc.scalar.dma_start(bk_sb[:], bk[0:128, :])
            nc.scalar.dma_start(bk_sb2[:], bk[128:192, :])
            for i in range(KT):
                nc.scalar.dma_start(wq_sb[i][:], wq[i * 128:(i + 1) * 128, :])
                nc.scalar.dma_start(wk_sb[i][:], wk[i * 128:(i + 1) * 128, :])
                nc.scalar.dma_start(wv_sb[i][:], wv[i * 128:(i + 1) * 128, :])
            # Q/K in [d, s] layout: heads 0,1 in tile0 (partitions 0-63 /
            # 64-127), head 2 in tile1 (partitions 0-63).  Same base
            # partition for q_j and k_j so the scores matmul operands align.
            q0 = constp.tile([128, S], att_dt, tag="q0")
            q1 = constp.tile([64, S], att_dt, tag="q1")
            k0 = constp.tile([128, S], att_dt, tag="k0")
            k1 = constp.tile([64, S], att_dt, tag="k1")
            # V' per head: [sk, 65] blocks stacked along free dim; col 64
            # stays 1.0 so PV also produces the softmax row-sums.
            vp = [constp.tile([128, SKT * (D + 1)], att_dt, tag=f"vp{j}", name=f"vp{j}")
                  for j in range(HPC)]
            for j in range(HPC):
                nc.vector.memset(vp[j][:], 1.0)

            # ---- phase 1a: qT / kT projection (+ bias, per-partition) ----
            with tc.tile_pool(name="psA", bufs=2, space="PSUM") as psA:
                for (dst, wsb, bsb, col0, m) in (
                    (q0, wq_sb, bq_sb, 0, 128),
                    (q1, wq_sb, bq_sb2, 128, 64),
                    (k0, wk_sb, bk_sb, 0, 128),
                    (k1, wk_sb, bk_sb2, 128, 64),
                ):
                    for c in range(SQC):
                        ps = psA.tile([m, 512], f32, tag=f"psA{m}", name=f"psA{m}")
                        for i in range(KT):
                            nc.tensor.matmul(
                                ps[:],
                                mmap(wsb[i][:, col0:col0 + m]),
                                mmap(ht[i][:, c * 512:(c + 1) * 512]),
                                start=(i == 0), stop=(i == KT - 1),
                            )
                        nc.vector.tensor_scalar_add(
                            dst[:, c * 512:(c + 1) * 512], ps[:], bsb[:])

                # ---- phase 1b: V in natural [s, d] layout ----
                for st in range(SKT):
                    psv = psA.tile([128, HPC * D], f32, tag="psV", name="psV")
                    for i in range(KT):
                        nc.tensor.matmul(
                            psv[:],
                            mmap(ht[i][:, st * 128:(st + 1) * 128]),
                            mmap(wv_sb[i][:]),
                            start=(i == 0), stop=(i == KT - 1),
                        )
                    for j in range(HPC):
                        nc.vector.tensor_copy(
                            vp[j][:, st * (D + 1):st * (D + 1) + D],
                            psv[:, j * D:(j + 1) * D])

            # ---- phase 2: attention per head ----
            qk_slices = (  # (q_ap, k_ap) per head, matching base partitions
                (q0[0:64, :], k0[0:64, :]),
                (q0[64:128, :], k0[64:128, :]),
                (q1[:, :], k1[:, :]),
            )
            with (
                tc.tile_pool(name="ps", bufs=2, space="PSUM") as psp,
                tc.tile_pool(name="po", bufs=4, space="PSUM") as pop,
            ):
                for j in range(HPC):
                    qap, kap = qk_slices[j]
                    po = [pop.tile([D + 1, 512], f32, tag="po", name=f"po{j}_{_c}") for _c in range(SQC)]
                    for st in range(SKT):
                        bt = biasp.tile([128, S], f16, name="bt")
                        dma_eng = (nc.sync, nc.scalar)[(j * SKT + st) % 2]
                        dma_eng.dma_start(
                            bt[:], bias_t[j, st * 128:(st + 1) * 128, :])
                        for half in range(2):
                            ps = psp.tile([128, 1024], f32, name="ps")
                            pt = ptp.tile([128, 1024], att_dt, name="pt")
                            for cc in range(2):
                                c = half * 2 + cc
                                sq = slice(c * 512, (c + 1) * 512)
                                if variant == "inject":
                                    nc.tensor.matmul(
                                        ps[:, cc * 512:(cc + 1) * 512],
                                        ident_sb[:],
                                        bt[:, sq],
                                        start=True, stop=False,
                                    )
                                nc.tensor.matmul(
                                    ps[:, cc * 512:(cc + 1) * 512],
                                    kap[:, st * 128:(st + 1) * 128],
                                    qap[:, sq],
                                    start=(variant != "inject"),
                                    stop=True,
                                )
                            if variant == "inject":
                                nc.scalar.activation(pt[:], ps[:], EXP)
                            else:
                                nc.scalar.activation(pt[:], ps[:], EXP)
                                nc.vector.tensor_mul(
                                    pt[:], pt[:],
                                    bt[:, half * 1024:(half + 1) * 1024])
                            for cc in range(2):
                                c = half * 2 + cc
                                nc.tensor.matmul(
                                    po[c][:],
                                    mmap(vp[j][:, st * (D + 1):(st + 1) * (D + 1)]),
                                    pt[:, cc * 512:(cc + 1) * 512],
                                    start=(st == 0), stop=(st == SKT - 1),
                                )
                    for c in range(SQC):
                        ot = otp.tile([D + 1, 512], f32, name="ot")
                        nc.vector.tensor_copy(ot[:], po[c][:])
                        nc.sync.dma_start(
                            out[j, :, c * 512:(c + 1) * 512], ot[:])

    nc.compile()
    return nc


def _get_nc(variant="inject"):
    if variant not in _CACHE:
        _CACHE[variant] = _build_nc(variant)
    return _CACHE[variant]


def _make_in_maps(hidden_states, Wqkv_w, Wqkv_b, bias, indices, variant="inject"):
    hidden_states = np.asarray(hidden_states, dtype=np.float32)
    Wqkv_w = np.asarray(Wqkv_w, dtype=np.float32)
    Wqkv_b = np.asarray(Wqkv_b, dtype=np.float32)
    bias = np.asarray(bias, dtype=np.float32)
    indices = np.asarray(indices, dtype=np.int64)

    scale = 1.0 / math.sqrt(D)
    padded = np.zeros((TOTAL, DIM), dtype=np.float32)
    padded[indices] = hidden_states

    Wq, Wk, Wv = Wqkv_w[0:DIM], Wqkv_w[DIM:2 * DIM], Wqkv_w[2 * DIM:3 * DIM]
    bq_full = Wqkv_b[0:DIM] * scale
    bk_full = Wqkv_b[DIM:2 * DIM]
    ident = np.eye(128, dtype=np.float16)

    in_maps = []
    for c in range(N_CORES):
        b = c // 4
        h0 = (c % 4) * HPC
        r = slice(h0 * D, (h0 + HPC) * D)
        bias_c = bias[b, h0:h0 + HPC].transpose(0, 2, 1)
        if variant == "expb":
            bias_c = np.exp(bias_c)
        in_maps.append({
            "hT": padded[b * S:(b + 1) * S].T.astype(np.float16),
            "wq": (Wq[r].T * np.float32(scale)).astype(np.float16),
            "wk": Wk[r].T.astype(np.float16),
            "wv": Wv[r].T.astype(np.float16),
            "bq": np.ascontiguousarray(bq_full[r].reshape(HPC * D, 1)),
            "bk": np.ascontiguousarray(bk_full[r].reshape(HPC * D, 1)),
            "bias_t": np.ascontiguousarray(bias_c.astype(np.float16)),
            "ident": ident,
        })
    return in_maps


def _assemble(results, Wqkv_b, indices):
    Wqkv_b = np.asarray(Wqkv_b, dtype=np.float32)
    indices = np.asarray(indices, dtype=np.int64)
    bv = Wqkv_b[2 * DIM:3 * DIM]
    out_full = np.empty((TOTAL, DIM), dtype=np.float32)
    for c in range(N_CORES):
        b = c // 4
        h0 = (c % 4) * HPC
        o = np.asarray(results[c]["out"], dtype=np.float32)  # (3, 65, 2048)
        for j in range(HPC):
            h = h0 + j
            att = (o[j, :D] / o[j, D]).T + bv[h * D:(h + 1) * D]
            out_full[b * S:(b + 1) * S, h * D:(h + 1) * D] = att
    return out_full[indices]


VARIANT = "expb"


def kernel(hidden_states, Wqkv_w, Wqkv_b, bias, slopes, cu_seqlens, indices,
           attn_mask, max_seqlen, **_unused):
    from concourse.bass_utils import run_bass_kernel_spmd

    nc = _get_nc(VARIANT)
    in_maps = _make_in_maps(hidden_states, Wqkv_w, Wqkv_b, bias, indices,
                            VARIANT)
    res = run_bass_kernel_spmd(nc, in_maps, list(range(N_CORES)))
    return _assemble(res.results, Wqkv_b, indices)



# revision 6
# speedup vs baseline: 1.0899x; 1.0899x over previous
"""Trainium2 Bass kernel for BertAlibiUnpadSelfAttention.

Problem shapes (hardcoded): B=2, S=2048, H=12, D=64, DIM=768.
Reference computation:
    qkv = hidden @ Wqkv_w.T + Wqkv_b            # (4096, 2304)
    pad via indices (a permutation -> pure row shuffle)
    q,k,v = split/reshape -> (b, h, s, d)
    scores = q @ k.T / sqrt(64) + bias          # bias dense (2,12,2048,2048)
    attn = softmax(scores) @ v -> (4096, 768), unpad via indices

Sharding: 24 (batch, head) pairs -> 3 per core across 8 cores.

v2 layout: single fused PE stream.  The attention loop is a 3-engine
software pipeline (PE scores -> ACT exp -> DVE mul -> PE PV) with the PV
lagging 2 blocks behind its QK so the PE never waits on the exp chain.
The q/k projection is interleaved into the attention stream as PE filler
work; only the V projection and the first q0/k0 chunks run up-front.
Blocks >= INJ_FROM add the bias in PSUM via an fp16 identity matmul
(denser PE stream, keeps HAM warm) instead of the DVE multiply.
"""

import math
import numpy as np

B, S, H, D = 2, 2048, 12, 64
DIM = H * D            # 768
TOTAL = B * S          # 4096
HPC = 3                # heads per core
N_CORES = 8
KT = DIM // 128        # 6 k-tiles of 128
SKT = S // 128         # 16 sk tiles of 128
NBLK = HPC * 2 * SKT   # 96 attention blocks per core

_CACHE = {}


def _inj(b, inj_from):
    return b >= inj_from


def _build_nc(inj_from=64):
    from concourse import bacc, mybir, tile

    f32 = mybir.dt.float32
    f16 = mybir.dt.float16

    nc = bacc.Bacc("TRN2", target_bir_lowering=False, debug=False)

    hT = nc.dram_tensor("hT", (DIM, S), f16, kind="ExternalInput")
    # host-packed weights: [p, (w*6+i)*192 + c] = W_w[i*128+p, c], w in q,k,v
    wqkv = nc.dram_tensor("wqkv", (128, 3 * KT * HPC * D), f16,
                          kind="ExternalInput")
    # host-packed projection biases: cols = [bq lo, bq hi, bk lo, bk hi]
    bvec = nc.dram_tensor("bvec", (128, 4), f32, kind="ExternalInput")
    # (j, sqh, c4, 128, 4096): host-rearranged bias; exp'd for expb blocks,
    # raw for inject blocks.
    bias_r = nc.dram_tensor("bias_r", (HPC, 2, 4, 128, 4096), f16,
                            kind="ExternalInput")
    ident = nc.dram_tensor("ident", (128, 128), f16, kind="ExternalInput")
    out = nc.dram_tensor("out", (HPC, 2, D + 1, 1024), f32,
                         kind="ExternalOutput")

    EXP = mybir.ActivationFunctionType.Exp

    with tile.TileContext(nc) as tc:
        with (
            tc.tile_pool(name="const", bufs=1) as constp,
            tc.tile_pool(name="bias", bufs=4) as biasp,
            tc.tile_pool(name="pt", bufs=4) as ptp,
            tc.tile_pool(name="ot", bufs=2) as otp,
        ):
            # ---- persistent SBUF tiles ----
            ht = [constp.tile([128, S], f16, tag=f"ht{i}", name=f"ht{i}")
                  for i in range(KT)]
            wqkv_sb = constp.tile([128, 3 * KT * HPC * D], f16, tag="wqkv")
            WB = HPC * D  # 192
            wq_sb = [wqkv_sb[:, (0 * KT + i) * WB:(0 * KT + i + 1) * WB] for i in range(KT)]
            wk_sb = [wqkv_sb[:, (1 * KT + i) * WB:(1 * KT + i + 1) * WB] for i in range(KT)]
            wv_sb = [wqkv_sb[:, (2 * KT + i) * WB:(2 * KT + i + 1) * WB] for i in range(KT)]
            bvec_sb = constp.tile([128, 4], f32, tag="bvec")
            bq_sb = bvec_sb[:, 0:1]
            bq_sb2 = bvec_sb[0:64, 1:2]
            bk_sb = bvec_sb[:, 2:3]
            bk_sb2 = bvec_sb[0:64, 3:4]
            ident_sb = constp.tile([128, 128], f16, tag="ident")
            # q/k in [d, s] layout; heads 0,1 stacked in q0/k0 partitions,
            # head 2 in q1/k1 (partitions 0-63).
            q0 = constp.tile([128, S], f16, tag="q0")
            q1 = constp.tile([64, S], f16, tag="q1")
            k0 = constp.tile([128, S], f16, tag="k0")
            k1 = constp.tile([64, S], f16, tag="k1")
            # V' per head: [sk, 65] blocks along free dim; col 64 stays 1.0
            # so PV also produces the softmax row-sums.
            vp = [constp.tile([128, SKT * (D + 1)], f16, tag=f"vp{j}", name=f"vp{j}")
                  for j in range(HPC)]

            # ---- input DMAs (all on the sync HWDGE ring; ACT queue stays
            # exp-only).  hT in column-piece order so early proj chunks
            # unblock sooner ----
            nc.sync.dma_start(wqkv_sb[:], wqkv[:, :])
            nc.sync.dma_start(bvec_sb[:], bvec[:, :])
            nc.sync.dma_start(ident_sb[:], ident[:, :])
            for p in range(2):
                for i in range(KT):
                    cs = slice(p * 1024, (p + 1) * 1024)
                    nc.sync.dma_start(ht[i][:, cs], hT[i * 128:(i + 1) * 128, cs])
            for j in range(HPC):
                nc.vector.memset(vp[j][:], 1.0)

            # ---- projection chunk helpers ----
            # q/k chunks: psum [m, 512] accumulated over KT, bias-added into
            # the persistent qT/kT tiles by DVE.
            def qk_chunk_units(pool, dst, wsb, bsb, col0, m, c):
                """Returns 3 callables, each emitting 2 matmuls (+TS on last)."""
                state = {}

                def unit(u):
                    def emit():
                        if u == 0:
                            state["ps"] = pool.tile([m, 512], f32, tag="pk",
                                                    name=f"pk{m}", bufs=2)
                        ps = state["ps"]
                        for i in (2 * u, 2 * u + 1):
                            nc.tensor.matmul(
                                ps[:],
                                wsb[i][:, col0:col0 + m],
                                ht[i][:, c * 512:(c + 1) * 512],
                                start=(i == 0), stop=(i == KT - 1),
                            )
                        if u == 2:
                            nc.vector.tensor_scalar_add(
                                dst[:, c * 512:(c + 1) * 512], ps[:], bsb[:])
                    return emit
                return [unit(0), unit(1), unit(2)]

            def v_tile(pool, st):
                def emit():
                    psv = pool.tile([128, HPC * D], f32, tag="pv", name="psv",
                                    bufs=2)
                    for i in range(KT):
                        nc.tensor.matmul(
                            psv[:],
                            ht[i][:, st * 128:(st + 1) * 128],
                            wv_sb[i][:],
                            start=(i == 0), stop=(i == KT - 1),
                        )
                    for j in range(HPC):
                        nc.vector.tensor_copy(
                            vp[j][:, st * (D + 1):st * (D + 1) + D],
                            psv[:, j * D:(j + 1) * D])
                return emit

            # ---- upfront: V (all heads) + q0 c0,c1 + k0 c0 ----
            with tc.tile_pool(name="projA", bufs=2, space="PSUM") as projp:
                for u in qk_chunk_units(projp, k0, wk_sb, bk_sb, 0, 128, 0):
                    u()
                for u in qk_chunk_units(projp, q0, wq_sb, bq_sb, 0, 128, 0):
                    u()
                for u in qk_chunk_units(projp, q0, wq_sb, bq_sb, 0, 128, 1):
                    u()
                for st in range(SKT):
                    v_tile(projp, st)()

            # ---- attention: 96-block pipeline with interleaved proj ----
            qk_slices = (
                (q0[0:64, :], k0[0:64, :]),
                (q0[64:128, :], k0[64:128, :]),
                (q1[:, :], k1[:, :]),
            )

            with tc.tile_pool(name="att", bufs=2, space="PSUM") as attp:
                # filler schedule: block -> list of emit callables
                filler = {b: [] for b in range(NBLK)}

                def sched_chunk(units, b0, stride):
                    for i, u in enumerate(units):
                        filler[b0 + i * stride].append(u)

                sched_chunk(qk_chunk_units(attp, k0, wk_sb, bk_sb, 0, 128, 1), 0, 1)
                sched_chunk(qk_chunk_units(attp, k0, wk_sb, bk_sb, 0, 128, 2), 3, 1)
                sched_chunk(qk_chunk_units(attp, k0, wk_sb, bk_sb, 0, 128, 3), 6, 1)
                sched_chunk(qk_chunk_units(attp, q0, wq_sb, bq_sb, 0, 128, 2), 9, 1)
                sched_chunk(qk_chunk_units(attp, q0, wq_sb, bq_sb, 0, 128, 3), 12, 1)
                b0 = 16
                for c in range(4):
                    for (dst, wsb, bsb) in ((q1, wq_sb, bq_sb2), (k1, wk_sb, bk_sb2)):
                        sched_chunk(qk_chunk_units(attp, dst, wsb, bsb, 128, 64, c),
                                    b0, 2)
                        b0 += 6

                blocks = [(j, sqh, st)
                          for j in range(HPC) for sqh in range(2)
                          for st in range(SKT)]
                pend = []            # PV lag queue: (j, sqh, st, pt, po)
                po_cur = {}
                bt_cur = {}

                def emit_pv(j, sqh, st, pt, po):
                    for cc in range(2):
                        nc.tensor.matmul(
                            po[:, cc * 512:(cc + 1) * 512],
                            vp[j][:, st * (D + 1):(st + 1) * (D + 1)],
                            pt[:, cc * 512:(cc + 1) * 512],
                            start=(st == 0), stop=(st == SKT - 1),
                        )
                    if st == SKT - 1:
                        ot = otp.tile([D + 1, 1024], f32, name="ot")
                        nc.vector.tensor_copy(ot[:], po[:])
                        nc.sync.dma_start(out[j, sqh], ot[:])

                for b, (j, sqh, st) in enumerate(blocks):
                    if st % 4 == 0:
                        bt = biasp.tile([128, 4096], f16, name="bt")
                        nc.sync.dma_start(bt[:], bias_r[j, sqh, st // 4])
                        bt_cur[(j, sqh)] = bt
                    bt = bt_cur[(j, sqh)]
                    boff = (st % 4) * 1024
                    if st == 0:
                        po_cur[(j, sqh)] = attp.tile([D + 1, 1024], f32,
                                                     tag="po", name="po", bufs=1)
                    qap, kap = qk_slices[j]
                    inj = _inj(b, inj_from)
                    ps = attp.tile([128, 1024], f32, tag="ps", name="ps", bufs=2)
                    if inj:
                        for cc in range(2):
                            nc.tensor.matmul(
                                ps[:, cc * 512:(cc + 1) * 512],
                                ident_sb[:],
                                bt[:, boff + cc * 512:boff + (cc + 1) * 512],
                                start=True, stop=False,
                            )
                    for cc in range(2):
                        nc.tensor.matmul(
                            ps[:, cc * 512:(cc + 1) * 512],
                            kap[:, st * 128:(st + 1) * 128],
                            qap[:, sqh * 1024 + cc * 512:sqh * 1024 + (cc + 1) * 512],
                            start=(not inj), stop=True,
                        )
                    for f in filler[b]:
                        f()
                    pt = ptp.tile([128, 1024], f16, name="pt")
                    nc.scalar.activation(pt[:], ps[:], EXP)
                    if not inj:
                        nc.vector.tensor_mul(pt[:], pt[:],
                                             bt[:, boff:boff + 1024])
                    pend.append((j, sqh, st, pt, po_cur[(j, sqh)]))
                    if len(pend) > 2:
                        emit_pv(*pend.pop(0))
                while pend:
                    emit_pv(*pend.pop(0))

    nc.compile()
    return nc


def _get_nc(inj_from=64):
    if inj_from not in _CACHE:
        _CACHE[inj_from] = _build_nc(inj_from)
    return _CACHE[inj_from]


def _make_in_maps(hidden_states, Wqkv_w, Wqkv_b, bias, indices, inj_from=64):
    hidden_states = np.asarray(hidden_states, dtype=np.float32)
    Wqkv_w = np.asarray(Wqkv_w, dtype=np.float32)
    Wqkv_b = np.asarray(Wqkv_b, dtype=np.float32)
    bias = np.asarray(bias, dtype=np.float32)
    indices = np.asarray(indices, dtype=np.int64)

    scale = 1.0 / math.sqrt(D)
    padded = np.zeros((TOTAL, DIM), dtype=np.float32)
    padded[indices] = hidden_states

    Wq, Wk, Wv = Wqkv_w[0:DIM], Wqkv_w[DIM:2 * DIM], Wqkv_w[2 * DIM:3 * DIM]
    bq_full = Wqkv_b[0:DIM] * scale
    bk_full = Wqkv_b[DIM:2 * DIM]
    ident = np.eye(128, dtype=np.float16)

    in_maps = []
    for c in range(N_CORES):
        b = c // 4
        h0 = (c % 4) * HPC
        r = slice(h0 * D, (h0 + HPC) * D)
        # bias_r[j, sqh, c4, p, st4*1024 + q] =
        #   f(bias[b, h0+j, sqh*1024 + q, (4*c4+st4)*128 + p])   (transposed)
        bias_c = bias[b, h0:h0 + HPC]                    # (3, sq, sk)
        bt = bias_c.transpose(0, 2, 1)                   # (3, sk, sq)
        bt = bt.reshape(HPC, 4, 4, 128, 2, 1024)         # (j, c4, st4, p, sqh, q)
        bt = bt.transpose(0, 4, 1, 3, 2, 5)              # (j, sqh, c4, p, st4, q)
        blk = (np.arange(HPC)[:, None, None, None] * 2 * SKT
               + np.arange(2)[None, :, None, None] * SKT
               + np.arange(4)[None, None, :, None] * 4
               + np.arange(4)[None, None, None, :])      # (j, sqh, c4, st4)
        expb = blk < inj_from
        bt = np.where(expb[:, :, :, None, :, None], np.exp(bt), bt)
        bias_r = np.ascontiguousarray(
            bt.reshape(HPC, 2, 4, 128, 4096).astype(np.float16))
        # pack q/k/v weight k-tiles: wqkv[p, (w*6+i)*192+c] = W.T[i*128+p, c]
        wt = np.stack([(Wq[r].T * np.float32(scale)), Wk[r].T, Wv[r].T])
        wqkv = np.ascontiguousarray(
            wt.reshape(3, KT, 128, HPC * D).transpose(2, 0, 1, 3)
            .reshape(128, 3 * KT * HPC * D).astype(np.float16))
        bvec = np.zeros((128, 4), dtype=np.float32)
        bvec[:, 0] = bq_full[r][0:128]
        bvec[0:64, 1] = bq_full[r][128:192]
        bvec[:, 2] = bk_full[r][0:128]
        bvec[0:64, 3] = bk_full[r][128:192]
        in_maps.append({
            "hT": padded[b * S:(b + 1) * S].T.astype(np.float16),
            "wqkv": wqkv,
            "bvec": bvec,
            "bias_r": bias_r,
            "ident": ident,
        })
    return in_maps


def _assemble(results, Wqkv_b, indices):
    Wqkv_b = np.asarray(Wqkv_b, dtype=np.float32)
    indices = np.asarray(indices, dtype=np.int64)
    bv = Wqkv_b[2 * DIM:3 * DIM]
    out_full = np.empty((TOTAL, DIM), dtype=np.float32)
    for c in range(N_CORES):
        b = c // 4
        h0 = (c % 4) * HPC
        o = np.asarray(results[c]["out"], dtype=np.float32)  # (3, 2, 65, 1024)
        for j in range(HPC):
            h = h0 + j
            oj = np.concatenate([o[j, 0], o[j, 1]], axis=1)  # (65, 2048)
            att = (oj[:D] / oj[D]).T + bv[h * D:(h + 1) * D]
            out_full[b * S:(b + 1) * S, h * D:(h + 1) * D] = att
    return out_full[indices]


INJ_FROM = 64


def kernel(hidden_states, Wqkv_w, Wqkv_b, bias, slopes, cu_seqlens, indices,
           attn_mask, max_seqlen, **_unused):
    from concourse.bass_utils import run_bass_kernel_spmd

    nc = _get_nc(INJ_FROM)
    in_maps = _make_in_maps(hidden_states, Wqkv_w, Wqkv_b, bias, indices,
                            INJ_FROM)
    res = run_bass_kernel_spmd(nc, in_maps, list(range(N_CORES)))
    return _assemble(res.results, Wqkv_b, indices)


# revision 18
# speedup vs baseline: 1.1196x; 1.0273x over previous
"""Trainium2 Bass kernel for BertAlibiUnpadSelfAttention.

Problem shapes (hardcoded): B=2, S=2048, H=12, D=64, DIM=768.
Reference computation:
    qkv = hidden @ Wqkv_w.T + Wqkv_b            # (4096, 2304)
    pad via indices (a permutation -> pure row shuffle)
    q,k,v = split/reshape -> (b, h, s, d)
    scores = q @ k.T / sqrt(64) + bias          # bias dense (2,12,2048,2048)
    attn = softmax(scores) @ v -> (4096, 768), unpad via indices

Sharding: 24 (batch, head) pairs -> 3 per core across 8 cores.

v2 layout: single fused PE stream.  The attention loop is a 3-engine
software pipeline (PE scores -> ACT exp -> DVE mul -> PE PV) with the PV
lagging 2 blocks behind its QK so the PE never waits on the exp chain.
The q/k projection is interleaved into the attention stream as PE filler
work; only the V projection and the first q0/k0 chunks run up-front.
Blocks >= INJ_FROM add the bias in PSUM via an fp16 identity matmul
(denser PE stream, keeps HAM warm) instead of the DVE multiply.
"""

import math
import numpy as np

B, S, H, D = 2, 2048, 12, 64
DIM = H * D            # 768
TOTAL = B * S          # 4096
HPC = 3                # heads per core
N_CORES = 8
KT = DIM // 128        # 6 k-tiles of 128
SKT = S // 128         # 16 sk tiles of 128
NBLK = HPC * 2 * SKT   # 96 attention blocks per core

_CACHE = {}


def _inj(b, inj_from):
    return b >= inj_from


def _build_nc(inj_from=64):
    from concourse import bacc, mybir, tile

    f32 = mybir.dt.float32
    f16 = mybir.dt.float16

    nc = bacc.Bacc("TRN2", target_bir_lowering=False, debug=False)

    hT = nc.dram_tensor("hT", (DIM, S), f16, kind="ExternalInput")
    # host-packed weights: [p, (w*6+i)*192 + c] = W_w[i*128+p, c], w in k,q,v
    wqkv = nc.dram_tensor("wqkv", (128, 3 * KT * HPC * D), f16,
                          kind="ExternalInput")
    # host-packed projection biases: cols = [bq lo, bq hi, bk lo, bk hi]
    bvec = nc.dram_tensor("bvec", (128, 4), f32, kind="ExternalInput")
    # (j, sqh, c4, 128, 4096): host-rearranged bias; exp'd for expb blocks,
    # raw for inject blocks.
    bias_r = nc.dram_tensor("bias_r", (HPC, 2, 4, 128, 4096), f16,
                            kind="ExternalInput")
    ident = nc.dram_tensor("ident", (128, 128), f16, kind="ExternalInput")
    out = nc.dram_tensor("out", (HPC, 2, D + 1, 1024), f32,
                         kind="ExternalOutput")

    EXP = mybir.ActivationFunctionType.Exp

    with tile.TileContext(nc) as tc:
        with (
            tc.tile_pool(name="const", bufs=1) as constp,
            tc.tile_pool(name="bias", bufs=4) as biasp,
            tc.tile_pool(name="pt", bufs=5) as ptp,
            tc.tile_pool(name="ot", bufs=2) as otp,
        ):
            # ---- persistent SBUF tiles ----
            ht = [constp.tile([128, S], f16, tag=f"ht{i}", name=f"ht{i}")
                  for i in range(KT)]
            wqkv_sb = constp.tile([128, 3 * KT * HPC * D], f16, tag="wqkv")
            WB = HPC * D  # 192
            wk_sb = [wqkv_sb[:, (0 * KT + i) * WB:(0 * KT + i + 1) * WB] for i in range(KT)]
            wq_sb = [wqkv_sb[:, (1 * KT + i) * WB:(1 * KT + i + 1) * WB] for i in range(KT)]
            wv_sb = [wqkv_sb[:, (2 * KT + i) * WB:(2 * KT + i + 1) * WB] for i in range(KT)]
            bvec_sb = constp.tile([128, 4], f32, tag="bvec")
            bq_sb = bvec_sb[:, 0:1]
            bq_sb2 = bvec_sb[0:64, 1:2]
            bk_sb = bvec_sb[:, 2:3]
            bk_sb2 = bvec_sb[0:64, 3:4]
            ident_sb = constp.tile([128, 128], f16, tag="ident")
            # q/k in [d, s] layout; heads 0,1 stacked in q0/k0 partitions,
            # head 2 in q1/k1 (partitions 0-63).
            q0 = constp.tile([128, S], f16, tag="q0")
            q1 = constp.tile([64, S], f16, tag="q1")
            k0 = constp.tile([128, S], f16, tag="k0")
            k1 = constp.tile([64, S], f16, tag="k1")
            # V' per head: [sk, 65] blocks along free dim; col 64 stays 1.0
            # so PV also produces the softmax row-sums.
            vp = [constp.tile([128, SKT * (D + 1)], f16, tag=f"vp{j}", name=f"vp{j}")
                  for j in range(HPC)]

            # ---- input DMAs (all on the sync HWDGE ring; ACT queue stays
            # exp-only).  Critical-path first: wk tiles, then hT column
            # pieces, so the first proj chunk unblocks ASAP ----
            nc.sync.dma_start(wqkv_sb[:, 0:KT * WB], wqkv[:, 0:KT * WB])
            for i in range(KT):
                nc.sync.dma_start(ht[i][:, 0:1024], hT[i * 128:(i + 1) * 128, 0:1024])
            nc.sync.dma_start(bvec_sb[:], bvec[:, :])
            nc.sync.dma_start(wqkv_sb[:, KT * WB:], wqkv[:, KT * WB:])
            for i in range(KT):
                nc.sync.dma_start(ht[i][:, 1024:2048], hT[i * 128:(i + 1) * 128, 1024:2048])
            nc.sync.dma_start(ident_sb[:], ident[:, :])
            for j in range(HPC):
                nc.vector.memset(vp[j][:], 1.0)
            # warmup source for the HAM pre-warm matmuls
            wsrc = constp.tile([64, 512], f16, tag="wsrc")
            nc.vector.memset(wsrc[:], 0.0)

            # ---- projection chunk helpers ----
            # q/k chunks: psum [m, 512] accumulated over KT, bias-added into
            # the persistent qT/kT tiles by DVE.
            def qk_chunk_units(pool, dst, wsb, bsb, col0, m, c, mm_per_unit=2):
                """Callables each emitting mm_per_unit matmuls (+TS on last)."""
                state = {}

                def unit(i0):
                    def emit():
                        if i0 == 0:
                            state["ps"] = pool.tile([m, 512], f32, tag="pk",
                                                    name=f"pk{m}", bufs=1)
                        ps = state["ps"]
                        for i in range(i0, i0 + mm_per_unit):
                            nc.tensor.matmul(
                                ps[:],
                                wsb[i][:, col0:col0 + m],
                                ht[i][:, c * 512:(c + 1) * 512],
                                start=(i == 0), stop=(i == KT - 1),
                            )
                        if i0 + mm_per_unit == KT:
                            nc.vector.tensor_scalar_add(
                                dst[:, c * 512:(c + 1) * 512], ps[:], bsb[:])
                    return emit
                return [unit(i0) for i0 in range(0, KT, mm_per_unit)]

            def v_tile(pool, st):
                def emit():
                    psv = pool.tile([128, HPC * D], f32, tag="pv", name="psv",
                                    bufs=1)
                    for i in range(KT):
                        nc.tensor.matmul(
                            psv[:],
                            ht[i][:, st * 128:(st + 1) * 128],
                            wv_sb[i][:],
                            start=(i == 0), stop=(i == KT - 1),
                        )
                    for j in range(HPC):
                        nc.vector.tensor_copy(
                            vp[j][:, st * (D + 1):st * (D + 1) + D],
                            psv[:, j * D:(j + 1) * D])
                return emit

            # ---- upfront: HAM pre-warm + k0/q0 c0 + q0 c1 + first V tiles ----
            with tc.tile_pool(name="projA", bufs=2, space="PSUM") as projp:
                warm_ps = projp.tile([64, 512], f32, tag="warm", bufs=1)
                for _ in range(14):
                    nc.tensor.matmul(warm_ps[:], wsrc[:, 0:64], wsrc[:],
                                     start=True, stop=True)
                for u in qk_chunk_units(projp, k0, wk_sb, bk_sb, 0, 128, 0):
                    u()
                for u in qk_chunk_units(projp, q0, wq_sb, bq_sb, 0, 128, 0):
                    u()
                for u in qk_chunk_units(projp, q0, wq_sb, bq_sb, 0, 128, 1):
                    u()
                for st in range(4):
                    v_tile(projp, st)()

            # ---- attention: 96-block pipeline with interleaved proj ----
            qk_slices = (
                (q0[0:64, :], k0[0:64, :]),
                (q0[64:128, :], k0[64:128, :]),
                (q1[:, :], k1[:, :]),
            )

            with tc.tile_pool(name="att", bufs=2, space="PSUM") as attp:
                # filler schedule: block -> list of emit callables
                filler = {b: [] for b in range(NBLK)}

                def sched_chunk(units, b0, stride):
                    for i, u in enumerate(units):
                        filler[b0 + i * stride].append(u)

                sched_chunk(qk_chunk_units(attp, k0, wk_sb, bk_sb, 0, 128, 1), 0, 1)
                sched_chunk(qk_chunk_units(attp, k0, wk_sb, bk_sb, 0, 128, 2), 3, 1)
                sched_chunk(qk_chunk_units(attp, k0, wk_sb, bk_sb, 0, 128, 3), 6, 1)
                sched_chunk(qk_chunk_units(attp, q0, wq_sb, bq_sb, 0, 128, 2), 9, 1)
                sched_chunk(qk_chunk_units(attp, q0, wq_sb, bq_sb, 0, 128, 3), 12, 1)
                # remaining V tiles, one per early block
                for st in range(4, SKT):
                    filler[st - 4].append(v_tile(attp, st))
                # head-2 q/k chunks: single-matmul units, one per block
                b0 = 15
                for c in range(4):
                    for (dst, wsb, bsb) in ((q1, wq_sb, bq_sb2), (k1, wk_sb, bk_sb2)):
                        sched_chunk(qk_chunk_units(attp, dst, wsb, bsb, 128, 64, c,
                                                   mm_per_unit=1), b0, 1)
                        b0 += 6

                blocks = [(j, sqh, st)
                          for j in range(HPC) for sqh in range(2)
                          for st in range(SKT)]
                pend = []            # PV lag queue: (j, sqh, st, pt, po)
                po_cur = {}
                bt_cur = {}

                def emit_pv(j, sqh, st, pt, po):
                    last = st == SKT - 1
                    ot = otp.tile([D + 1, 1024], f32, name="ot") if last else None
                    for cc in range(2):
                        nc.tensor.matmul(
                            po[:, cc * 512:(cc + 1) * 512],
                            vp[j][:, st * (D + 1):(st + 1) * (D + 1)],
                            pt[:, cc * 512:(cc + 1) * 512],
                            start=(st == 0), stop=last,
                        )
                        if last:
                            nc.vector.tensor_copy(
                                ot[:, cc * 512:(cc + 1) * 512],
                                po[:, cc * 512:(cc + 1) * 512])
                    if last:
                        nc.sync.dma_start(out[j, sqh], ot[:])

                for b, (j, sqh, st) in enumerate(blocks):
                    if st % 4 == 0:
                        bt = biasp.tile([128, 4096], f16, name="bt")
                        nc.sync.dma_start(bt[:], bias_r[j, sqh, st // 4])
                        bt_cur[(j, sqh)] = bt
                    bt = bt_cur[(j, sqh)]
                    boff = (st % 4) * 1024
                    if st == 0:
                        po_cur[(j, sqh)] = attp.tile([D + 1, 1024], f32,
                                                     tag="po", name="po", bufs=1)
                    qap, kap = qk_slices[j]
                    inj = _inj(b, inj_from)
                    ps = attp.tile([128, 1024], f32, tag="ps", name="ps", bufs=2)
                    if inj:
                        for cc in range(2):
                            nc.tensor.matmul(
                                ps[:, cc * 512:(cc + 1) * 512],
                                ident_sb[:],
                                bt[:, boff + cc * 512:boff + (cc + 1) * 512],
                                start=True, stop=False,
                            )
                    for cc in range(2):
                        nc.tensor.matmul(
                            ps[:, cc * 512:(cc + 1) * 512],
                            kap[:, st * 128:(st + 1) * 128],
                            qap[:, sqh * 1024 + cc * 512:sqh * 1024 + (cc + 1) * 512],
                            start=(not inj), stop=True,
                        )
                    for f in filler[b]:
                        f()
                    pt = ptp.tile([128, 1024], f16, name="pt")
                    nc.scalar.activation(pt[:], ps[:], EXP)
                    if not inj:
                        nc.vector.tensor_mul(pt[:], pt[:],
                                             bt[:, boff:boff + 1024])
                    pend.append((j, sqh, st, pt, po_cur[(j, sqh)]))
                    if len(pend) > 3:
                        emit_pv(*pend.pop(0))
                while pend:
                    emit_pv(*pend.pop(0))

    nc.compile()
    return nc


def _get_nc(inj_from=64):
    if inj_from not in _CACHE:
        _CACHE[inj_from] = _build_nc(inj_from)
    return _CACHE[inj_from]


def _make_in_maps(hidden_states, Wqkv_w, Wqkv_b, bias, indices, inj_from=64):
    hidden_states = np.asarray(hidden_states, dtype=np.float32)
    Wqkv_w = np.asarray(Wqkv_w, dtype=np.float32)
    Wqkv_b = np.asarray(Wqkv_b, dtype=np.float32)
    bias = np.asarray(bias, dtype=np.float32)
    indices = np.asarray(indices, dtype=np.int64)

    scale = 1.0 / math.sqrt(D)
    padded = np.zeros((TOTAL, DIM), dtype=np.float32)
    padded[indices] = hidden_states

    Wq, Wk, Wv = Wqkv_w[0:DIM], Wqkv_w[DIM:2 * DIM], Wqkv_w[2 * DIM:3 * DIM]
    bq_full = Wqkv_b[0:DIM] * scale
    bk_full = Wqkv_b[DIM:2 * DIM]
    ident = np.eye(128, dtype=np.float16)

    in_maps = []
    for c in range(N_CORES):
        b = c // 4
        h0 = (c % 4) * HPC
        r = slice(h0 * D, (h0 + HPC) * D)
        # bias_r[j, sqh, c4, p, st4*1024 + q] =
        #   f(bias[b, h0+j, sqh*1024 + q, (4*c4+st4)*128 + p])   (transposed)
        bias_c = bias[b, h0:h0 + HPC]                    # (3, sq, sk)
        bt = bias_c.transpose(0, 2, 1)                   # (3, sk, sq)
        bt = bt.reshape(HPC, 4, 4, 128, 2, 1024)         # (j, c4, st4, p, sqh, q)
        bt = bt.transpose(0, 4, 1, 3, 2, 5)              # (j, sqh, c4, p, st4, q)
        blk = (np.arange(HPC)[:, None, None, None] * 2 * SKT
               + np.arange(2)[None, :, None, None] * SKT
               + np.arange(4)[None, None, :, None] * 4
               + np.arange(4)[None, None, None, :])      # (j, sqh, c4, st4)
        expb = blk < inj_from
        bt = np.where(expb[:, :, :, None, :, None], np.exp(bt), bt)
        bias_r = np.ascontiguousarray(
            bt.reshape(HPC, 2, 4, 128, 4096).astype(np.float16))
        # pack k/q/v weight k-tiles: wqkv[p, (w*6+i)*192+c] = W.T[i*128+p, c]
        wt = np.stack([Wk[r].T, (Wq[r].T * np.float32(scale)), Wv[r].T])
        wqkv = np.ascontiguousarray(
            wt.reshape(3, KT, 128, HPC * D).transpose(2, 0, 1, 3)
            .reshape(128, 3 * KT * HPC * D).astype(np.float16))
        bvec = np.zeros((128, 4), dtype=np.float32)
        bvec[:, 0] = bq_full[r][0:128]
        bvec[0:64, 1] = bq_full[r][128:192]
        bvec[:, 2] = bk_full[r][0:128]
        bvec[0:64, 3] = bk_full[r][128:192]
        in_maps.append({
            "hT": padded[b * S:(b + 1) * S].T.astype(np.float16),
            "wqkv": wqkv,
            "bvec": bvec,
            "bias_r": bias_r,
            "ident": ident,
        })
    return in_maps


def _assemble(results, Wqkv_b, indices):
    Wqkv_b = np.asarray(Wqkv_b, dtype=np.float32)
    indices = np.asarray(indices, dtype=np.int64)
    bv = Wqkv_b[2 * DIM:3 * DIM]
    out_full = np.empty((TOTAL, DIM), dtype=np.float32)
    for c in range(N_CORES):
        b = c // 4
        h0 = (c % 4) * HPC
        o = np.asarray(results[c]["out"], dtype=np.float32)  # (3, 2, 65, 1024)
        for j in range(HPC):
            h = h0 + j
            oj = np.concatenate([o[j, 0], o[j, 1]], axis=1)  # (65, 2048)
            att = (oj[:D] / oj[D]).T + bv[h * D:(h + 1) * D]
            out_full[b * S:(b + 1) * S, h * D:(h + 1) * D] = att
    return out_full[indices]


INJ_FROM = 96


def kernel(hidden_states, Wqkv_w, Wqkv_b, bias, slopes, cu_seqlens, indices,
           attn_mask, max_seqlen, **_unused):
    from concourse.bass_utils import run_bass_kernel_spmd

    nc = _get_nc(INJ_FROM)
    in_maps = _make_in_maps(hidden_states, Wqkv_w, Wqkv_b, bias, indices,
                            INJ_FROM)
    res = run_bass_kernel_spmd(nc, in_maps, list(range(N_CORES)))
    return _assemble(res.results, Wqkv_b, indices)


# revision 23
# speedup vs baseline: 1.3992x; 1.2497x over previous
"""Trainium2 Bass kernel for BertAlibiUnpadSelfAttention.

Problem shapes (hardcoded): B=2, S=2048, H=12, D=64, DIM=768.
Reference computation:
    qkv = hidden @ Wqkv_w.T + Wqkv_b            # (4096, 2304)
    pad via indices (a permutation -> pure row shuffle)
    q,k,v = split/reshape -> (b, h, s, d)
    scores = q @ k.T / sqrt(64) + bias          # bias dense (2,12,2048,2048)
    attn = softmax(scores) @ v -> (4096, 768), unpad via indices

Sharding: 24 (batch, head) pairs -> 3 per core across 8 cores.

v2 layout: single fused PE stream.  The attention loop is a 3-engine
software pipeline (PE scores -> ACT exp -> DVE mul -> PE PV) with the PV
lagging 2 blocks behind its QK so the PE never waits on the exp chain.
The q/k projection is interleaved into the attention stream as PE filler
work; only the V projection and the first q0/k0 chunks run up-front.
Blocks >= INJ_FROM add the bias in PSUM via an fp16 identity matmul
(denser PE stream, keeps HAM warm) instead of the DVE multiply.
"""

import math
import numpy as np

B, S, H, D = 2, 2048, 12, 64
DIM = H * D            # 768
TOTAL = B * S          # 4096
HPC = 3                # heads per core
N_CORES = 8
KT = DIM // 128        # 6 k-tiles of 128
SKT = S // 128         # 16 sk tiles of 128
NBLK = HPC * 2 * SKT   # 96 attention blocks per core

_CACHE = {}


def _inj(b, inj_from):
    return b >= inj_from


def _build_nc(inj_from=64):
    from concourse import bacc, mybir, tile

    f32 = mybir.dt.float32
    f16 = mybir.dt.float16

    nc = bacc.Bacc("TRN2", target_bir_lowering=False, debug=False)

    hT = nc.dram_tensor("hT", (DIM, S), f16, kind="ExternalInput")
    # host-packed weights: [p, (w*6+i)*192 + c] = W_w[i*128+p, c], w in k,q,v
    wqkv = nc.dram_tensor("wqkv", (128, 3 * KT * HPC * D), f16,
                          kind="ExternalInput")
    # host-packed projection biases: cols = [bq lo, bq hi, bk lo, bk hi]
    bvec = nc.dram_tensor("bvec", (128, 4), f32, kind="ExternalInput")
    # (j, sqh, c4, 128, 4096): host-rearranged bias; exp'd for expb blocks,
    # raw for inject blocks.
    bias_r = nc.dram_tensor("bias_r", (HPC, 2, 4, 128, 4096), f16,
                            kind="ExternalInput")
    ident = nc.dram_tensor("ident", (128, 128), f16, kind="ExternalInput")
    out = nc.dram_tensor("out", (HPC, 2, D + 1, 1024), f32,
                         kind="ExternalOutput")

    EXP = mybir.ActivationFunctionType.Exp

    with tile.TileContext(nc) as tc:
        with (
            tc.tile_pool(name="const", bufs=1) as constp,
            tc.tile_pool(name="bias", bufs=4) as biasp,
            tc.tile_pool(name="pt", bufs=5) as ptp,
            tc.tile_pool(name="ot", bufs=2) as otp,
        ):
            # ---- persistent SBUF tiles ----
            ht = [constp.tile([128, S], f16, tag=f"ht{i}", name=f"ht{i}")
                  for i in range(KT)]
            wqkv_sb = constp.tile([128, 3 * KT * HPC * D], f16, tag="wqkv")
            WB = HPC * D  # 192
            wk_sb = [wqkv_sb[:, (0 * KT + i) * WB:(0 * KT + i + 1) * WB] for i in range(KT)]
            wq_sb = [wqkv_sb[:, (1 * KT + i) * WB:(1 * KT + i + 1) * WB] for i in range(KT)]
            wv_sb = [wqkv_sb[:, (2 * KT + i) * WB:(2 * KT + i + 1) * WB] for i in range(KT)]
            bvec_sb = constp.tile([128, 4], f32, tag="bvec")
            bq_sb = bvec_sb[:, 0:1]
            bq_sb2 = bvec_sb[0:64, 1:2]
            bk_sb = bvec_sb[:, 2:3]
            bk_sb2 = bvec_sb[0:64, 3:4]
            ident_sb = constp.tile([128, 128], f16, tag="ident")
            # q/k in [d, s] layout; heads 0,1 stacked in q0/k0 partitions,
            # head 2 in q1/k1 (partitions 0-63).
            q0 = constp.tile([128, S], f16, tag="q0")
            q1 = constp.tile([64, S], f16, tag="q1")
            k0 = constp.tile([128, S], f16, tag="k0")
            k1 = constp.tile([64, S], f16, tag="k1")
            # V' per head: [sk, 65] blocks along free dim; col 64 stays 1.0
            # so PV also produces the softmax row-sums.
            vp = [constp.tile([128, SKT * (D + 1)], f16, tag=f"vp{j}", name=f"vp{j}")
                  for j in range(HPC)]

            # ---- input DMAs ride the scalar HWDGE ring (ACT queue is idle
            # until the first exp anyway); bias gets the sync ring from t=0.
            # Critical-path order: wk+wq tiles, then hT in 512-column pieces
            # so the first proj chunks unblock ASAP ----
            wsrc = constp.tile([64, 512], f16, tag="wsrc")
            nc.vector.memset(wsrc[:], 0.0)
            nc.scalar.dma_start(wqkv_sb[:, 0:2 * KT * WB], wqkv[:, 0:2 * KT * WB])
            for c in range(4):
                cs = slice(c * 512, (c + 1) * 512)
                for i in range(KT):
                    nc.scalar.dma_start(ht[i][:, cs], hT[i * 128:(i + 1) * 128, cs])
                if c == 0:
                    nc.scalar.dma_start(bvec_sb[:], bvec[:, :])
                elif c == 1:
                    nc.scalar.dma_start(wqkv_sb[:, 2 * KT * WB:], wqkv[:, 2 * KT * WB:])
                elif c == 2:
                    nc.scalar.dma_start(ident_sb[:], ident[:, :])
            for j in range(HPC):
                nc.vector.memset(vp[j][:], 1.0)

            # ---- projection chunk helpers ----
            # q/k chunks: psum [m, 512] accumulated over KT, bias-added into
            # the persistent qT/kT tiles by DVE.
            def qk_chunk(pool, dst, wsb, bsb, col0, m, c, tag="pk", bufs=2):
                """Emit one full projection chunk: 6 matmuls + bias-add."""
                def emit():
                    ps = pool.tile([m, 512], f32, tag=tag, name=f"pk{m}",
                                   bufs=bufs)
                    for i in range(KT):
                        nc.tensor.matmul(
                            ps[:],
                            wsb[i][:, col0:col0 + m],
                            ht[i][:, c * 512:(c + 1) * 512],
                            start=(i == 0), stop=(i == KT - 1),
                        )
                    nc.vector.tensor_scalar_add(
                        dst[:, c * 512:(c + 1) * 512], ps[:], bsb[:])
                return emit

            def v_tile(pool, st, tag="pv", bufs=2):
                def emit():
                    psv = pool.tile([128, HPC * D], f32, tag=tag, name="psv",
                                    bufs=bufs)
                    for i in range(KT):
                        nc.tensor.matmul(
                            psv[:],
                            ht[i][:, st * 128:(st + 1) * 128],
                            wv_sb[i][:],
                            start=(i == 0), stop=(i == KT - 1),
                        )
                    for j in range(HPC):
                        nc.vector.tensor_copy(
                            vp[j][:, st * (D + 1):st * (D + 1) + D],
                            psv[:, j * D:(j + 1) * D])
                return emit

            # ---- upfront: HAM pre-warm + k0/q0 c0 + q0 c1 + first V tiles ----
            with tc.tile_pool(name="projA", bufs=2, space="PSUM") as projp:
                warm_ps = projp.tile([64, 64], f32, tag="warm", bufs=1)
                for _ in range(60):
                    nc.tensor.matmul(warm_ps[:], wsrc[:, 0:64], wsrc[:, 0:64],
                                     start=True, stop=True)
                qk_chunk(projp, k0, wk_sb, bk_sb, 0, 128, 0)()
                qk_chunk(projp, q0, wq_sb, bq_sb, 0, 128, 0)()
                qk_chunk(projp, q0, wq_sb, bq_sb, 0, 128, 1)()
                for st in range(6):
                    v_tile(projp, st)()

            # ---- attention: 96-block pipeline with interleaved proj ----
            qk_slices = (
                (q0[0:64, :], k0[0:64, :]),
                (q0[64:128, :], k0[64:128, :]),
                (q1[:, :], k1[:, :]),
            )

            with tc.tile_pool(name="att", bufs=3, space="PSUM") as attp:
                # filler schedule: block -> one full proj unit (a chunk or a
                # V tile), borrowing a "ps"-tag PSUM slot for its lifetime
                filler = {b: [] for b in range(NBLK)}

                def aux_chunk(dst, wsb, bsb, col0, m, c):
                    return qk_chunk(attp, dst, wsb, bsb, col0, m, c,
                                    tag="ps", bufs=3)

                filler[0].append(aux_chunk(k0, wk_sb, bk_sb, 0, 128, 1))
                filler[3].append(aux_chunk(k0, wk_sb, bk_sb, 0, 128, 2))
                filler[6].append(aux_chunk(k0, wk_sb, bk_sb, 0, 128, 3))
                filler[10].append(aux_chunk(q0, wq_sb, bq_sb, 0, 128, 2))
                filler[13].append(aux_chunk(q0, wq_sb, bq_sb, 0, 128, 3))
                vslots = [1, 2, 4, 5, 7, 8, 9, 11, 12, 14]
                for st, b in zip(range(6, SKT), vslots):
                    filler[b].append(v_tile(attp, st, tag="ps", bufs=3))
                # head-2 q/k chunks, one per ~6 blocks
                b0 = 16
                for c in range(4):
                    for (dst, wsb, bsb) in ((q1, wq_sb, bq_sb2), (k1, wk_sb, bk_sb2)):
                        filler[b0].append(aux_chunk(dst, wsb, bsb, 128, 64, c))
                        b0 += 6

                blocks = [(j, sqh, st)
                          for j in range(HPC) for sqh in range(2)
                          for st in range(SKT)]
                pend = []            # PV lag queue: (j, sqh, st, pt, po)
                po_cur = {}
                bt_cur = {}

                def emit_pv(j, sqh, st, pt, po):
                    last = st == SKT - 1
                    ot = otp.tile([D + 1, 1024], f32, name="ot") if last else None
                    for cc in range(2):
                        nc.tensor.matmul(
                            po[:, cc * 512:(cc + 1) * 512],
                            vp[j][:, st * (D + 1):(st + 1) * (D + 1)],
                            pt[:, cc * 512:(cc + 1) * 512],
                            start=(st == 0), stop=last,
                        )
                        if last:
                            nc.vector.tensor_copy(
                                ot[:, cc * 512:(cc + 1) * 512],
                                po[:, cc * 512:(cc + 1) * 512])
                    if last:
                        nc.sync.dma_start(out[j, sqh], ot[:])

                for b, (j, sqh, st) in enumerate(blocks):
                    if st % 4 == 0:
                        bt = biasp.tile([128, 4096], f16, name="bt")
                        nc.sync.dma_start(bt[:], bias_r[j, sqh, st // 4])
                        bt_cur[(j, sqh)] = bt
                    bt = bt_cur[(j, sqh)]
                    boff = (st % 4) * 1024
                    if st == 0:
                        po_cur[(j, sqh)] = attp.tile([D + 1, 1024], f32,
                                                     tag="po", name="po", bufs=1)
                    qap, kap = qk_slices[j]
                    inj = _inj(b, inj_from)
                    ps = attp.tile([128, 1024], f32, tag="ps", name="ps", bufs=3)
                    if inj:
                        for cc in range(2):
                            nc.tensor.matmul(
                                ps[:, cc * 512:(cc + 1) * 512],
                                ident_sb[:],
                                bt[:, boff + cc * 512:boff + (cc + 1) * 512],
                                start=True, stop=False,
                            )
                    for cc in range(2):
                        nc.tensor.matmul(
                            ps[:, cc * 512:(cc + 1) * 512],
                            kap[:, st * 128:(st + 1) * 128],
                            qap[:, sqh * 1024 + cc * 512:sqh * 1024 + (cc + 1) * 512],
                            start=(not inj), stop=True,
                        )
                    for f in filler[b]:
                        f()
                    pt = ptp.tile([128, 1024], f16, name="pt")
                    nc.scalar.activation(pt[:], ps[:], EXP)
                    if not inj:
                        nc.vector.tensor_mul(pt[:], pt[:],
                                             bt[:, boff:boff + 1024])
                    pend.append((j, sqh, st, pt, po_cur[(j, sqh)]))
                    if len(pend) > 3:
                        emit_pv(*pend.pop(0))
                while pend:
                    emit_pv(*pend.pop(0))

    nc.compile()
    return nc


def _get_nc(inj_from=64):
    if inj_from not in _CACHE:
        _CACHE[inj_from] = _build_nc(inj_from)
    return _CACHE[inj_from]


def _make_in_maps(hidden_states, Wqkv_w, Wqkv_b, bias, indices, inj_from=64):
    hidden_states = np.asarray(hidden_states, dtype=np.float32)
    Wqkv_w = np.asarray(Wqkv_w, dtype=np.float32)
    Wqkv_b = np.asarray(Wqkv_b, dtype=np.float32)
    bias = np.asarray(bias, dtype=np.float32)
    indices = np.asarray(indices, dtype=np.int64)

    scale = 1.0 / math.sqrt(D)
    padded = np.zeros((TOTAL, DIM), dtype=np.float32)
    padded[indices] = hidden_states

    Wq, Wk, Wv = Wqkv_w[0:DIM], Wqkv_w[DIM:2 * DIM], Wqkv_w[2 * DIM:3 * DIM]
    bq_full = Wqkv_b[0:DIM] * scale
    bk_full = Wqkv_b[DIM:2 * DIM]
    ident = np.eye(128, dtype=np.float16)

    in_maps = []
    for c in range(N_CORES):
        b = c // 4
        h0 = (c % 4) * HPC
        r = slice(h0 * D, (h0 + HPC) * D)
        # bias_r[j, sqh, c4, p, st4*1024 + q] =
        #   f(bias[b, h0+j, sqh*1024 + q, (4*c4+st4)*128 + p])   (transposed)
        bias_c = bias[b, h0:h0 + HPC]                    # (3, sq, sk)
        bt = bias_c.transpose(0, 2, 1)                   # (3, sk, sq)
        bt = bt.reshape(HPC, 4, 4, 128, 2, 1024)         # (j, c4, st4, p, sqh, q)
        bt = bt.transpose(0, 4, 1, 3, 2, 5)              # (j, sqh, c4, p, st4, q)
        blk = (np.arange(HPC)[:, None, None, None] * 2 * SKT
               + np.arange(2)[None, :, None, None] * SKT
               + np.arange(4)[None, None, :, None] * 4
               + np.arange(4)[None, None, None, :])      # (j, sqh, c4, st4)
        expb = blk < inj_from
        bt = np.where(expb[:, :, :, None, :, None], np.exp(bt), bt)
        bias_r = np.ascontiguousarray(
            bt.reshape(HPC, 2, 4, 128, 4096).astype(np.float16))
        # pack k/q/v weight k-tiles: wqkv[p, (w*6+i)*192+c] = W.T[i*128+p, c]
        wt = np.stack([Wk[r].T, (Wq[r].T * np.float32(scale)), Wv[r].T])
        wqkv = np.ascontiguousarray(
            wt.reshape(3, KT, 128, HPC * D).transpose(2, 0, 1, 3)
            .reshape(128, 3 * KT * HPC * D).astype(np.float16))
        bvec = np.zeros((128, 4), dtype=np.float32)
        bvec[:, 0] = bq_full[r][0:128]
        bvec[0:64, 1] = bq_full[r][128:192]
        bvec[:, 2] = bk_full[r][0:128]
        bvec[0:64, 3] = bk_full[r][128:192]
        in_maps.append({
            "hT": padded[b * S:(b + 1) * S].T.astype(np.float16),
            "wqkv": wqkv,
            "bvec": bvec,
            "bias_r": bias_r,
            "ident": ident,
        })
    return in_maps


def _assemble(results, Wqkv_b, indices):
    Wqkv_b = np.asarray(Wqkv_b, dtype=np.float32)
    indices = np.asarray(indices, dtype=np.int64)
    bv = Wqkv_b[2 * DIM:3 * DIM]
    out_full = np.empty((TOTAL, DIM), dtype=np.float32)
    for c in range(N_CORES):
        b = c // 4
        h0 = (c % 4) * HPC
        o = np.asarray(results[c]["out"], dtype=np.float32)  # (3, 2, 65, 1024)
        for j in range(HPC):
            h = h0 + j
            oj = np.concatenate([o[j, 0], o[j, 1]], axis=1)  # (65, 2048)
            att = (oj[:D] / oj[D]).T + bv[h * D:(h + 1) * D]
            out_full[b * S:(b + 1) * S, h * D:(h + 1) * D] = att
    return out_full[indices]


INJ_FROM = 96


def kernel(hidden_states, Wqkv_w, Wqkv_b, bias, slopes, cu_seqlens, indices,
           attn_mask, max_seqlen, **_unused):
    from concourse.bass_utils import run_bass_kernel_spmd

    nc = _get_nc(INJ_FROM)
    in_maps = _make_in_maps(hidden_states, Wqkv_w, Wqkv_b, bias, indices,
                            INJ_FROM)
    res = run_bass_kernel_spmd(nc, in_maps, list(range(N_CORES)))
    return _assemble(res.results, Wqkv_b, indices)
